# revision 22
# baseline (speedup 1.0000x reference)
"""Trainium2 Bass kernel for the attention-LSTM captioning RNN.

Problem (per full batch): x(64,128,512), A(64,1024,4,4), Wx(512,4096),
Wh(1024,4096), Wattn(1024,4096), b(4096) -> h-sequence (64,128,1024).

Strategy: data-parallel over N across 8 cores (8 samples/core, weights
replicated).  Per core, recurrence in transposed ("a^T") layout: gates
live on 128 partitions (partition = gate-col % 128), batch (8) on the
free dim.  Weights are the stationary operands (FWL), h^T the 8-wide
moving operands.

v5 pipeline summary:
  - x@Wx precomputed for ALL timesteps in a one-time setup GEMM
    (N=512-wide moving -> near-roofline), stored to internal DRAM as
    bf16, and streamed back per step into a 4-deep SBUF ring.  Each
    step injects its xW slice into PSUM via an identity-stationary
    matmul (start=True), so the recurrence has NO grouped x batching.
  - per-step PSUM gate bank (ring of 4 full banks): no cross-step bank
    hazard; next step's Wh writes never serialize behind this step's
    PSUM-reading activations.
  - gate columns permuted host-side to quarter order [i, g, f, o]:
    the Tensor stream runs i+g chunks first, then f, then o, so the
    cell chain (t2 -> u -> c -> tanh(c)) overlaps the remaining gate
    matmuls and h is ready ~1.5us after the last o-fold.
  - zsp/rz/ee emitted mid-Wh so softmax normalization hides under the
    Wh rounds; folds follow immediately.
  - scores emitted first (unblocks the softmax Taylor chain ASAP).
"""

import math
import sys

sys.path.insert(0, "/root/shim")
sys.path.insert(0, "/opt/trn_rl_repo")

import numpy as np
import ml_dtypes

try:
    import antenv

    if "/root/shim/antenv" not in list(antenv.__path__):
        antenv.__path__.append("/root/shim/antenv")
except Exception:
    pass

import concourse.bass as bass
import concourse.bacc as bacc
import concourse.mybir as mybir
from concourse.tile import TileContext
from concourse.bass_utils import run_bass_kernel_spmd

FP32 = mybir.dt.float32
BF16 = mybir.dt.bfloat16

# Problem constants (hardcoded per harness contract)
N, T, D, H = 64, 128, 512, 1024
NC = 8            # cores
NL = N // NC      # samples per core = 8
G = 4 * H         # 4096 gate columns
L = 16            # attention locations
HC = H // 128     # 8 h-chunks
GM = G // 128     # 32 gate-col chunks
DC = D // 128     # 4 d-chunks
INV_SQRT_H = 1.0 / math.sqrt(H)

UNROLL = 16       # steps per hardware-loop body
NB = 4            # per-step PSUM bank ring depth (also xw SBUF ring)

ADD = mybir.AluOpType.add
MULT = mybir.AluOpType.mult
TANH = mybir.ActivationFunctionType.Tanh
ET = mybir.EngineType

# quarter order after host-side column permutation: [i, g, f, o]
# m-chunks:  i = 0..7, g = 8..15, f = 16..23, o = 24..31


def build_nc(timesteps=T):
    nc = bacc.Bacc()

    # ---- DRAM I/O (host-prepped layouts) ----
    xT_d = nc.dram_tensor("xT", [128, DC, NL, timesteps], BF16, kind="ExternalInput")
    afT_d = nc.dram_tensor("afT", [128, HC, NL, L], FP32, kind="ExternalInput")
    wx_d = nc.dram_tensor("wx", [128, DC, G], BF16, kind="ExternalInput")
    wh_d = nc.dram_tensor("wh", [128, HC, G], BF16, kind="ExternalInput")
    wattn_d = nc.dram_tensor("wattn", [128, HC, G], BF16, kind="ExternalInput")
    bp_d = nc.dram_tensor("biasp", [128, G], BF16, kind="ExternalInput")
    mask_d = nc.dram_tensor("mask", [128, NL], FP32, kind="ExternalInput")
    bmask_d = nc.dram_tensor("bmask", [128, 128], FP32, kind="ExternalInput")
    ident_d = nc.dram_tensor("ident", [128, 128], BF16, kind="ExternalInput")
    out_d = nc.dram_tensor("hsT", [timesteps, 128, HC, NL], BF16, kind="ExternalOutput")
    # precomputed x@Wx, one [128, GM*NL] slice per step (+pad for prefetch)
    xw_d = nc.dram_tensor("xwpre", [timesteps + NB, 128, GM * NL], BF16, kind="Internal")

    TH = timesteps // 2  # t-half for the setup GEMM (PSUM 512-col limit)

    with TileContext(nc) as tc:
        with tc.tile_pool(name="persist", bufs=1) as pp:
            afTb = pp.tile([128, HC, NL, L], BF16)     # Af^T bf16 scaled 1/256
            p_sb = pp.tile([128, G], BF16)             # P[(n,l), g]
            wh_sb = pp.tile([128, HC, G], BF16)        # Wh tiles
            mask_sb = pp.tile([128, NL], FP32)
            bmask_sb = pp.tile([128, 128], FP32)       # 16-block partition mask
            ident_sb = pp.tile([128, 128], BF16)       # identity (xW inject)
            # h^T double-buffered AND split in two hc-halves so next step's
            # Wh kc0-3 can start as soon as the first half of h exists
            hTbs = [
                [
                    pp.tile([128, HC // 2, NL], BF16, name=f"hTb{i}{ha}")
                    for ha in range(2)
                ]
                for i in range(4)
            ]
            cT = pp.tile([128, HC, NL], FP32)          # 2c convention
            xwr = [
                pp.tile([128, GM, NL], BF16, name=f"xwr{i}") for i in range(NB)
            ]                                          # per-step xW ring

            nc.sync.dma_start(mask_sb[:], mask_d[:])
            nc.sync.dma_start(bmask_sb[:], bmask_d[:])
            nc.sync.dma_start(ident_sb[:], ident_d[:])

            # ---------- setup 1: xW[t] = x_t @ Wx for all t ----------
            with (
                tc.tile_pool(name="setup_x", bufs=1) as sxp,
                tc.tile_pool(name="stgp", bufs=2) as stgp,
                tc.tile_pool(name="xwpsum", bufs=3, space="PSUM") as xpp,
            ):
                xsb = sxp.tile([128, DC, NL, timesteps], BF16)
                wx_sb = sxp.tile([128, DC, G], BF16)
                nc.sync.dma_start(xsb[:], xT_d[:])
                nc.sync.dma_start(wx_sb[:], wx_d[:])
                # wh is only needed once the recurrence starts; queue it
                # behind the setup-GEMM inputs
                nc.sync.dma_start(wh_sb[:], wh_d[:])
                for th in range(2):
                    stg = stgp.tile([128, TH, GM, NL], BF16, tag="stg")
                    for m in range(GM):
                        # moving operand streams t-contiguous (fast); the
                        # (n,t)->(t,n) transpose happens in the DVE drain
                        ps = xpp.tile([128, NL, TH], FP32, tag="xps")
                        for kc in range(DC):
                            nc.tensor.matmul(
                                ps[:],
                                wx_sb[:, kc, m * 128:(m + 1) * 128],
                                xsb[:, kc, :, th * TH:(th + 1) * TH],
                                start=(kc == 0),
                                stop=(kc == DC - 1),
                            )
                        nc.vector.tensor_copy(
                            stg[:, :, m, :], ps[:].rearrange("p n t -> p t n")
                        )
                    nc.sync.dma_start(
                        xw_d[bass.ds(th * TH, TH), :, :].rearrange(
                            "t p c -> p t c"
                        ),
                        stg[:].rearrange("p t m n -> p t (m n)"),
                    )

            # ---------- setup 2: P = Af^T @ Wattn  (bf16, one-time) ----------
            with (
                tc.tile_pool(name="setup_a", bufs=1) as sap,
                tc.tile_pool(name="wsl", bufs=2) as wslp,
                tc.tile_pool(name="ppsum", bufs=1, space="PSUM") as ppp,
            ):
                afT = sap.tile([128, HC, NL, L], FP32)
                nc.sync.dma_start(afT[:], afT_d[:])
                bp_sb = sap.tile([128, G], BF16)
                nc.sync.dma_start(bp_sb[:], bp_d[:])
                # afTb scaled 1/256: s_col becomes u = (s/32)/4 directly
                nc.vector.tensor_scalar_mul(afTb[:], afT[:], 1.0 / 256.0)
                afTr = sap.tile([128, HC, NL, L], BF16)  # unscaled, for P
                nc.vector.tensor_copy(afTr[:], afT[:])

                # h0 = mean over l of Af; cT = 2*c0, hTb = 2*h0
                nc.vector.tensor_reduce(
                    cT[:], afT[:], axis=mybir.AxisListType.X,
                    op=mybir.AluOpType.add,
                )
                nc.vector.tensor_scalar_mul(cT[:], cT[:], 2.0 / L)
                nc.vector.tensor_copy(hTbs[0][0][:], cT[:, 0:HC // 2, :])
                nc.vector.tensor_copy(hTbs[0][1][:], cT[:, HC // 2:, :])

                pps = [
                    ppp.tile([128, 1024], FP32, tag=f"pps{gc}", name=f"pps{gc}")
                    for gc in range(4)
                ]
                for hc in range(HC):
                    wsl = wslp.tile([128, G], BF16, tag="wsl")
                    nc.sync.dma_start(wsl[:], wattn_d[:, hc, :])
                    for gc in range(4):
                        for hf in range(2):
                            nc.tensor.matmul(
                                pps[gc][:, hf * 512:(hf + 1) * 512],
                                afTr[:, hc, :, :].rearrange("p n l -> p (n l)"),
                                wsl[
                                    :,
                                    gc * 1024 + hf * 512:gc * 1024 + (hf + 1) * 512,
                                ],
                                start=(hc == 0),
                                stop=(hc == HC - 1),
                            )
                # P += bias: softmax weights sum to 1, so folding b into
                # every (n,l) row of P injects exactly b per step
                for gc in range(4):
                    nc.vector.tensor_tensor(
                        p_sb[:, gc * 1024:(gc + 1) * 1024],
                        pps[gc][:],
                        bp_sb[:, gc * 1024:(gc + 1) * 1024],
                        ADD,
                    )

            # preload xw ring for steps 0, 1
            for i in range(2):
                nc.sync.dma_start(
                    xwr[i][:].rearrange("p m n -> p (m n)"),
                    xw_d[bass.ds(i, 1), :, :].rearrange("t p c -> p (t c)"),
                )

            # ---------- recurrence ----------
            with (
                tc.tile_pool(name="step", bufs=2) as sp,
                tc.tile_pool(name="gpsum", bufs=1, space="PSUM") as gp,
                tc.tile_pool(name="spsum", bufs=1, space="PSUM") as ssp,
                tc.tile_pool(name="zpsum", bufs=1, space="PSUM") as zzp,
            ):
                # per-quarter-group gate banks (one PSUM bank each, ring-1:
                # each quarter's ACT read completes mid-step, so the next
                # step's inject WAR never binds).  m-chunks: IG=0..15,
                # F=16..23, Oa=24..27, Ob=28..31.
                sbIG = gp.tile([128, 16, NL], FP32, tag="sbIG", name="sbIG")
                sbF = gp.tile([128, 8, NL], FP32, tag="sbF", name="sbF")
                sbOa = gp.tile([128, 4, NL], FP32, tag="sbOa", name="sbOa")
                sbOb = gp.tile([128, 4, NL], FP32, tag="sbOb", name="sbOb")
                scp = ssp.tile([128, NL], FP32, tag="scp", name="scp")
                zsp = zzp.tile([128, 1], FP32, tag="zsp", name="zsp")
                H2 = HC // 2

                def mm(out, lhsT, rhs, start=False, stop=False):
                    nc.tensor.matmul(
                        out, lhsT, rhs,
                        start=start, stop=stop, skip_group_check=True,
                    )

                def sb_ap(m):
                    """gate-bank AP for m-chunk m -> (tile, local index)"""
                    if m < 16:
                        return sbIG[:, m, :]
                    if m < 24:
                        return sbF[:, m - 16, :]
                    if m < 28:
                        return sbOa[:, m - 24, :]
                    return sbOb[:, m - 28, :]

                def step(ti, slot, hin, hout):
                    hinA, hinB = hin

                    # ----- prefetch xw for step s+2 into the ring -----
                    nc.sync.dma_start(
                        xwr[(slot + 2) % NB][:].rearrange("p m n -> p (m n)"),
                        xw_d[bass.ds(ti + 2, 1), :, :].rearrange(
                            "t p c -> p (t c)"
                        ),
                    )

                    # ----- PE: xW inject (start=True clears each bank) -----
                    xw = xwr[slot]
                    mm(
                        sbIG[:].rearrange("p m n -> p (m n)"), ident_sb[:],
                        xw[:, 0:16, :].rearrange("p m n -> p (m n)"),
                        start=True,
                    )
                    mm(
                        sbF[:].rearrange("p m n -> p (m n)"), ident_sb[:],
                        xw[:, 16:24, :].rearrange("p m n -> p (m n)"),
                        start=True,
                    )
                    mm(
                        sbOa[:].rearrange("p m n -> p (m n)"), ident_sb[:],
                        xw[:, 24:28, :].rearrange("p m n -> p (m n)"),
                        start=True,
                    )
                    mm(
                        sbOb[:].rearrange("p m n -> p (m n)"), ident_sb[:],
                        xw[:, 28:32, :].rearrange("p m n -> p (m n)"),
                        start=True,
                    )

                    def hin_kc(kc):
                        return hinA[:, kc, :] if kc < H2 else hinB[:, kc - H2, :]

                    # ----- PE: scores (unblocks softmax ASAP) -----
                    for kc in range(HC):
                        nc.tensor.matmul(
                            scp[:],
                            afTb[:, kc, :, :].rearrange("p n l -> p (n l)"),
                            hin_kc(kc),
                            start=(kc == 0),
                            stop=(kc == HC - 1),
                        )

                    # ----- softmax chain at high scheduler priority: each op
                    # pops the moment it's ready, so the PE sem updates land
                    # right after scores/zsp instead of after the Wh burst
                    with tc.high_priority():
                        # V: diagonal-block extract s_col
                        junk = sp.tile([128, NL], FP32, tag="junk")
                        s_col = sp.tile([128, 1], FP32, tag="s_col")
                        nc.vector.scalar_tensor_tensor(
                            junk[:], scp[:], 1.0, mask_sb[:],
                            MULT, MULT,
                            accum_out=s_col[:],
                        )
                        # ACT: e^s = exp(4*u) in one Scalar op
                        e_col = sp.tile([128, 1], FP32, tag="e_col")
                        nc.scalar.activation(
                            e_col[:], s_col[:],
                            mybir.ActivationFunctionType.Exp, scale=4.0,
                        )


                    def wh_m(ms, kcs):
                        # kc-major: consecutive MMs accumulate into DIFFERENT
                        # PSUM addresses (same-address back-to-back accumulate
                        # costs ~4ns/MM in the drain path)
                        for kc in kcs:
                            for m in ms:
                                mm(
                                    sb_ap(m),
                                    wh_sb[:, kc, m * 128:(m + 1) * 128],
                                    hin_kc(kc),
                                )

                    def folds(ms):
                        for m in ms:
                            mm(
                                sb_ap(m),
                                p_sb[:, m * 128:(m + 1) * 128],
                                ee[:],
                                stop=True,
                            )

                    # i + g quarters: kc halves split so the first block only
                    # needs the first half of h from the previous step
                    wh_m(range(0, 16), range(0, H2))
                    # zsp placed here: ~2.5us into the Wh stream, safely after
                    # e_col resolves (~1.3us) -> never stalls the in-order PE
                    # queue, and ee is ready long before the folds' slot.
                    nc.tensor.matmul(
                        zsp[:], bmask_sb[:], e_col[:],
                        start=True, stop=True,
                    )
                    with tc.high_priority():
                        rz = sp.tile([128, 1], FP32, tag="rz")
                        nc.vector.reciprocal(rz[:], zsp[:])
                        # ee = (mask * e_col) * (1/z)
                        ee = sp.tile([128, NL], BF16, tag="ee")
                        nc.vector.tensor_scalar(
                            ee[:], mask_sb[:], e_col[:, 0:1], rz[:, 0:1],
                            MULT, MULT,
                        )
                    wh_m(range(0, 16), range(H2, HC))
                    folds(range(0, 16))
                    folds(range(16, 24))
                    folds(range(24, 28))
                    folds(range(28, 32))

                    # ----- tail, pipelined by quarter group -----
                    glIG = sp.tile([128, 2, HC, NL], FP32, tag="glIG")
                    nc.scalar.activation(
                        glIG[:],
                        sbIG[:].rearrange("p (q c) n -> p q c n", q=2),
                        TANH,
                    )
                    gi = glIG[:, 0]
                    gg = glIG[:, 1]
                    # t2 = (gl_i + 1) * gl_g = 2 i g
                    t2 = sp.tile([128, HC, NL], FP32, tag="t2")
                    nc.vector.scalar_tensor_tensor(t2[:], gi, 1.0, gg, ADD, MULT)

                    wh_m(range(16, 24), range(0, HC))
                    glF = sp.tile([128, HC, NL], FP32, tag="glF")
                    nc.scalar.activation(glF[:], sbF[:], TANH)
                    cs = cT[:, :, :]
                    # u = (gl_f + 1) * cT(2c) = 4 f c
                    u = sp.tile([128, HC, NL], FP32, tag="u")
                    nc.vector.scalar_tensor_tensor(u[:], glF[:], 1.0, cs, ADD, MULT)
                    # cT = 0.5*u + t2 = 2 c_new
                    nc.vector.scalar_tensor_tensor(cs, u[:], 0.5, t2[:], MULT, ADD)
                    # tct = tanh(0.5 * cT) = tanh(c)   (before glO on Scalar)
                    tct = sp.tile([128, HC, NL], FP32, tag="tct")
                    nc.scalar.activation(tct[:], cs, TANH, scale=0.5)

                    # o quarter, split in two hc-halves: h2a lands early so
                    # the next step's Wh kc0-3 can start under this tail
                    houtA, houtB = hout
                    wh_m(range(24, 28), range(0, HC))
                    glOa = sp.tile([128, H2, NL], FP32, tag="glOa")
                    nc.scalar.activation(glOa[:], sbOa[:], TANH)
                    nc.vector.scalar_tensor_tensor(
                        houtA[:], glOa[:], 1.0, tct[:, 0:H2, :], ADD, MULT
                    )
                    wh_m(range(28, 32), range(0, HC))
                    glOb = sp.tile([128, H2, NL], FP32, tag="glOb")
                    nc.scalar.activation(glOb[:], sbOb[:], TANH)
                    nc.vector.scalar_tensor_tensor(
                        houtB[:], glOb[:], 1.0, tct[:, H2:, :], ADD, MULT
                    )
                    nc.sync.dma_start(
                        out_d[bass.ds(ti, 1), :, 0:H2, :].rearrange(
                            "t p c n -> p (t c) n"
                        ),
                        houtA[:],
                    )
                    nc.sync.dma_start(
                        out_d[bass.ds(ti, 1), :, H2:, :].rearrange(
                            "t p c n -> p (t c) n"
                        ),
                        houtB[:],
                    )

                with tc.For_i(
                    0, timesteps, UNROLL,
                    staggered_reset=True,
                    hint_engines=(ET.PE, ET.DVE, ET.SP, ET.Activation),
                ) as ti0:
                    for s in range(UNROLL):
                        step(
                            ti0 + s, s % NB,
                            hTbs[s % 4], hTbs[(s + 1) % 4],
                        )

    nc.finalize()
    return nc


def prep_inputs(x, A, Wx, Wh, Wattn, b):
    """Host-side reshapes to device layouts; returns per-core input maps."""
    x = np.asarray(x, dtype=np.float32)
    A = np.asarray(A, dtype=np.float32)
    Wx = np.asarray(Wx, dtype=np.float32)
    Wh = np.asarray(Wh, dtype=np.float32)
    Wattn = np.asarray(Wattn, dtype=np.float32)
    b = np.asarray(b, dtype=np.float32)
    timesteps = x.shape[1]

    # permute gate columns to quarter order [i, g, f, o]
    gperm = np.concatenate([
        np.arange(0, H),          # i
        np.arange(3 * H, 4 * H),  # g
        np.arange(H, 2 * H),      # f
        np.arange(2 * H, 3 * H),  # o
    ])
    Wx = Wx[:, gperm]
    Wh = Wh[:, gperm]
    Wattn = Wattn[:, gperm]
    b = b[gperm]

    # weight layouts [p, kc, g] with k = kc*128 + p
    # per-gate-column scaling: i/f/o columns carry a 0.5 (tanh half-angle
    # trick), g columns stay full-scale; Wh gets an extra 0.5 (h2 = 2h).
    gsc = np.ones((G,), np.float32) * 0.5
    gsc[H:2 * H] = 1.0            # g quarter (permuted position)
    whs = (0.5 * gsc) * Wh
    wxs = gsc * Wx
    wh_h = np.ascontiguousarray(
        whs.reshape(HC, 128, G).transpose(1, 0, 2).astype(ml_dtypes.bfloat16)
    )
    wx_h = np.ascontiguousarray(
        wxs.reshape(DC, 128, G).transpose(1, 0, 2).astype(ml_dtypes.bfloat16)
    )
    wattn_h = np.ascontiguousarray(
        (gsc * Wattn).reshape(HC, 128, G).transpose(1, 0, 2).astype(
            ml_dtypes.bfloat16
        )
    )
    # bias replicated across partitions; folded into P on device
    bp_h = np.ascontiguousarray(
        np.broadcast_to(gsc * b, (128, G)).astype(ml_dtypes.bfloat16)
    )
    mask_h = np.zeros((128, NL), dtype=np.float32)
    for p in range(128):
        mask_h[p, p // L] = 1.0
    bmask_h = (
        np.arange(128)[:, None] // L == np.arange(128)[None, :] // L
    ).astype(np.float32)
    ident_h = np.eye(128, dtype=np.float32).astype(ml_dtypes.bfloat16)

    in_maps = []
    for c in range(NC):
        xs = x[c * NL:(c + 1) * NL]          # (8, T, 512)
        As = A[c * NL:(c + 1) * NL].reshape(NL, H, L)  # (8, 1024, 16)
        # xT [p, dc, n, t] = x[n, t, dc*128+p]
        xT_h = np.ascontiguousarray(
            xs.reshape(NL, timesteps, DC, 128).transpose(3, 2, 0, 1)
            .astype(ml_dtypes.bfloat16)
        )
        # afT [p, hc, n, l] = Af[n, hc*128+p, l]
        afT_h = np.ascontiguousarray(
            As.reshape(NL, HC, 128, L).transpose(2, 1, 0, 3)
        )
        in_maps.append(
            {
                "xT": xT_h,
                "afT": afT_h,
                "wx": wx_h,
                "wh": wh_h,
                "wattn": wattn_h,
                "biasp": bp_h,
                "mask": mask_h,
                "bmask": bmask_h,
                "ident": ident_h,
            }
        )
    return in_maps


_NC_CACHE = {}


def kernel(x, A, Wx, Wh, Wattn, b, trace=False):
    timesteps = x.shape[1]
    key = timesteps
    if key not in _NC_CACHE:
        _NC_CACHE[key] = build_nc(timesteps)
    nc = _NC_CACHE[key]
    in_maps = prep_inputs(x, A, Wx, Wh, Wattn, b)
    res = run_bass_kernel_spmd(nc, in_maps, list(range(NC)), trace=trace)
    outs = []
    for c in range(NC):
        hsT = res.results[c]["hsT"]  # (T, 128, HC, NL)
        # out[n, t, hc*128+p] = hsT[t, p, hc, n]
        outs.append(
            0.5
            * hsT.astype(np.float32).transpose(3, 0, 2, 1).reshape(
                NL, timesteps, H
            )
        )
    full = np.concatenate(outs, axis=0).astype(np.float32)
    kernel.last_result = res
    return full


# revision 23
# speedup vs baseline: 1.0363x; 1.0363x over previous
"""Trainium2 Bass kernel for the attention-LSTM captioning RNN.

Problem (per full batch): x(64,128,512), A(64,1024,4,4), Wx(512,4096),
Wh(1024,4096), Wattn(1024,4096), b(4096) -> h-sequence (64,128,1024).

Strategy: data-parallel over N across 8 cores (8 samples/core, weights
replicated).  Per core, recurrence in transposed ("a^T") layout: gates
live on 128 partitions (partition = gate-col % 128), batch (8) on the
free dim.  Weights are the stationary operands (FWL), h^T the 8-wide
moving operands.

v5 pipeline summary:
  - x@Wx precomputed for ALL timesteps in a one-time setup GEMM
    (N=512-wide moving -> near-roofline), stored to internal DRAM as
    bf16, and streamed back per step into a 4-deep SBUF ring.  Each
    step injects its xW slice into PSUM via an identity-stationary
    matmul (start=True), so the recurrence has NO grouped x batching.
  - per-step PSUM gate bank (ring of 4 full banks): no cross-step bank
    hazard; next step's Wh writes never serialize behind this step's
    PSUM-reading activations.
  - gate columns permuted host-side to quarter order [i, g, f, o]:
    the Tensor stream runs i+g chunks first, then f, then o, so the
    cell chain (t2 -> u -> c -> tanh(c)) overlaps the remaining gate
    matmuls and h is ready ~1.5us after the last o-fold.
  - zsp/rz/ee emitted mid-Wh so softmax normalization hides under the
    Wh rounds; folds follow immediately.
  - scores emitted first (unblocks the softmax Taylor chain ASAP).
"""

import math
import sys

sys.path.insert(0, "/root/shim")
sys.path.insert(0, "/opt/trn_rl_repo")

import numpy as np
import ml_dtypes

try:
    import antenv

    if "/root/shim/antenv" not in list(antenv.__path__):
        antenv.__path__.append("/root/shim/antenv")
except Exception:
    pass

import concourse.bass as bass
import concourse.bacc as bacc
import concourse.mybir as mybir
from concourse.tile import TileContext
from concourse.bass_utils import run_bass_kernel_spmd

FP32 = mybir.dt.float32
BF16 = mybir.dt.bfloat16

# Problem constants (hardcoded per harness contract)
N, T, D, H = 64, 128, 512, 1024
NC = 8            # cores
NL = N // NC      # samples per core = 8
G = 4 * H         # 4096 gate columns
L = 16            # attention locations
HC = H // 128     # 8 h-chunks
GM = G // 128     # 32 gate-col chunks
DC = D // 128     # 4 d-chunks
INV_SQRT_H = 1.0 / math.sqrt(H)

UNROLL = 32       # steps per hardware-loop body
NB = 4            # per-step PSUM bank ring depth (also xw SBUF ring)

ADD = mybir.AluOpType.add
MULT = mybir.AluOpType.mult
TANH = mybir.ActivationFunctionType.Tanh
ET = mybir.EngineType

# quarter order after host-side column permutation: [i, g, f, o]
# m-chunks:  i = 0..7, g = 8..15, f = 16..23, o = 24..31


def build_nc(timesteps=T):
    nc = bacc.Bacc()

    # ---- DRAM I/O (host-prepped layouts) ----
    xT_d = nc.dram_tensor("xT", [128, DC, NL, timesteps], BF16, kind="ExternalInput")
    afT_d = nc.dram_tensor("afT", [128, HC, NL, L], FP32, kind="ExternalInput")
    wx_d = nc.dram_tensor("wx", [128, DC, G], BF16, kind="ExternalInput")
    wh_d = nc.dram_tensor("wh", [128, HC, G], BF16, kind="ExternalInput")
    wattn_d = nc.dram_tensor("wattn", [128, HC, G], BF16, kind="ExternalInput")
    bp_d = nc.dram_tensor("biasp", [128, G], BF16, kind="ExternalInput")
    mask_d = nc.dram_tensor("mask", [128, NL], FP32, kind="ExternalInput")
    bmask_d = nc.dram_tensor("bmask", [128, 128], BF16, kind="ExternalInput")
    ident_d = nc.dram_tensor("ident", [128, 128], BF16, kind="ExternalInput")
    out_d = nc.dram_tensor("hsT", [timesteps, 128, HC, NL], BF16, kind="ExternalOutput")
    # precomputed x@Wx, one [128, GM*NL] slice per step (+pad for prefetch)
    xw_d = nc.dram_tensor("xwpre", [timesteps + NB, 128, GM * NL], BF16, kind="Internal")

    TH = timesteps // 2  # t-half for the setup GEMM (PSUM 512-col limit)

    with TileContext(nc) as tc:
        with tc.tile_pool(name="persist", bufs=1) as pp:
            afTb = pp.tile([128, HC, NL, L], BF16)     # Af^T bf16 scaled 1/256
            p_sb = pp.tile([128, G], BF16)             # P[(n,l), g]
            wh_sb = pp.tile([128, HC, G], BF16)        # Wh tiles
            mask_sb = pp.tile([128, NL], FP32)
            bmask_sb = pp.tile([128, 128], BF16)       # 16-block partition mask
            ident_sb = pp.tile([128, 128], BF16)       # identity (xW inject)
            # h^T double-buffered AND split in two hc-halves so next step's
            # Wh kc0-3 can start as soon as the first half of h exists
            hTbs = [
                [
                    pp.tile([128, HC // 2, NL], BF16, name=f"hTb{i}{ha}")
                    for ha in range(2)
                ]
                for i in range(4)
            ]
            cT = pp.tile([128, HC, NL], FP32)          # 2c convention
            xwr = [
                pp.tile([128, GM, NL], BF16, name=f"xwr{i}") for i in range(NB)
            ]                                          # per-step xW ring

            nc.sync.dma_start(mask_sb[:], mask_d[:])
            nc.sync.dma_start(bmask_sb[:], bmask_d[:])
            nc.sync.dma_start(ident_sb[:], ident_d[:])

            # ---------- setup 1: xW[t] = x_t @ Wx for all t ----------
            with (
                tc.tile_pool(name="setup_x", bufs=1) as sxp,
                tc.tile_pool(name="stgp", bufs=2) as stgp,
                tc.tile_pool(name="xwpsum", bufs=3, space="PSUM") as xpp,
            ):
                xsb = sxp.tile([128, DC, NL, timesteps], BF16)
                wx_sb = sxp.tile([128, DC, G], BF16)
                nc.sync.dma_start(xsb[:], xT_d[:])
                nc.sync.dma_start(wx_sb[:], wx_d[:])
                # wh is only needed once the recurrence starts; queue it
                # behind the setup-GEMM inputs
                nc.sync.dma_start(wh_sb[:], wh_d[:])
                for th in range(2):
                    stg = stgp.tile([128, TH, GM, NL], BF16, tag="stg")
                    for m in range(GM):
                        # moving operand streams t-contiguous (fast); the
                        # (n,t)->(t,n) transpose happens in the DVE drain
                        ps = xpp.tile([128, NL, TH], FP32, tag="xps")
                        for kc in range(DC):
                            nc.tensor.matmul(
                                ps[:],
                                wx_sb[:, kc, m * 128:(m + 1) * 128],
                                xsb[:, kc, :, th * TH:(th + 1) * TH],
                                start=(kc == 0),
                                stop=(kc == DC - 1),
                            )
                        nc.vector.tensor_copy(
                            stg[:, :, m, :], ps[:].rearrange("p n t -> p t n")
                        )
                    nc.sync.dma_start(
                        xw_d[bass.ds(th * TH, TH), :, :].rearrange(
                            "t p c -> p t c"
                        ),
                        stg[:].rearrange("p t m n -> p t (m n)"),
                    )

            # ---------- setup 2: P = Af^T @ Wattn  (bf16, one-time) ----------
            with (
                tc.tile_pool(name="setup_a", bufs=1) as sap,
                tc.tile_pool(name="wsl", bufs=2) as wslp,
                tc.tile_pool(name="ppsum", bufs=1, space="PSUM") as ppp,
            ):
                afT = sap.tile([128, HC, NL, L], FP32)
                nc.sync.dma_start(afT[:], afT_d[:])
                bp_sb = sap.tile([128, G], BF16)
                nc.sync.dma_start(bp_sb[:], bp_d[:])
                # afTb scaled 1/256: s_col becomes u = (s/32)/4 directly
                nc.vector.tensor_scalar_mul(afTb[:], afT[:], 1.0 / 256.0)
                afTr = sap.tile([128, HC, NL, L], BF16)  # unscaled, for P
                nc.vector.tensor_copy(afTr[:], afT[:])

                # h0 = mean over l of Af; cT = 2*c0, hTb = 2*h0
                nc.vector.tensor_reduce(
                    cT[:], afT[:], axis=mybir.AxisListType.X,
                    op=mybir.AluOpType.add,
                )
                nc.vector.tensor_scalar_mul(cT[:], cT[:], 2.0 / L)
                nc.vector.tensor_copy(hTbs[0][0][:], cT[:, 0:HC // 2, :])
                nc.vector.tensor_copy(hTbs[0][1][:], cT[:, HC // 2:, :])

                pps = [
                    ppp.tile([128, 1024], FP32, tag=f"pps{gc}", name=f"pps{gc}")
                    for gc in range(4)
                ]
                for hc in range(HC):
                    wsl = wslp.tile([128, G], BF16, tag="wsl")
                    nc.sync.dma_start(wsl[:], wattn_d[:, hc, :])
                    for gc in range(4):
                        for hf in range(2):
                            nc.tensor.matmul(
                                pps[gc][:, hf * 512:(hf + 1) * 512],
                                afTr[:, hc, :, :].rearrange("p n l -> p (n l)"),
                                wsl[
                                    :,
                                    gc * 1024 + hf * 512:gc * 1024 + (hf + 1) * 512,
                                ],
                                start=(hc == 0),
                                stop=(hc == HC - 1),
                            )
                # P += bias: softmax weights sum to 1, so folding b into
                # every (n,l) row of P injects exactly b per step
                for gc in range(4):
                    nc.vector.tensor_tensor(
                        p_sb[:, gc * 1024:(gc + 1) * 1024],
                        pps[gc][:],
                        bp_sb[:, gc * 1024:(gc + 1) * 1024],
                        ADD,
                    )

            # preload xw ring for steps 0, 1
            for i in range(2):
                nc.sync.dma_start(
                    xwr[i][:].rearrange("p m n -> p (m n)"),
                    xw_d[bass.ds(i, 1), :, :].rearrange("t p c -> p (t c)"),
                )

            # ---------- recurrence ----------
            with (
                tc.tile_pool(name="step", bufs=2) as sp,
                tc.tile_pool(name="gpsum", bufs=1, space="PSUM") as gp,
                tc.tile_pool(name="spsum", bufs=1, space="PSUM") as ssp,
                tc.tile_pool(name="zpsum", bufs=1, space="PSUM") as zzp,
            ):
                # per-quarter-group gate banks (one PSUM bank each, ring-1:
                # each quarter's ACT read completes mid-step, so the next
                # step's inject WAR never binds).  m-chunks: IG=0..15,
                # F=16..23, Oa=24..27, Ob=28..31.
                sbIG = gp.tile([128, 16, NL], FP32, tag="sbIG", name="sbIG")
                sbF = gp.tile([128, 8, NL], FP32, tag="sbF", name="sbF")
                sbOa = gp.tile([128, 4, NL], FP32, tag="sbOa", name="sbOa")
                sbOb = gp.tile([128, 4, NL], FP32, tag="sbOb", name="sbOb")
                scp = ssp.tile([128, NL], FP32, tag="scp", name="scp")
                zsp = zzp.tile([128, 1], FP32, tag="zsp", name="zsp")
                H2 = HC // 2

                def mm(out, lhsT, rhs, start=False, stop=False):
                    nc.tensor.matmul(
                        out, lhsT, rhs,
                        start=start, stop=stop, skip_group_check=True,
                    )

                def sb_ap(m):
                    """gate-bank AP for m-chunk m -> (tile, local index)"""
                    if m < 16:
                        return sbIG[:, m, :]
                    if m < 24:
                        return sbF[:, m - 16, :]
                    if m < 28:
                        return sbOa[:, m - 24, :]
                    return sbOb[:, m - 28, :]

                def step(ti, slot, hin, hout):
                    hinA, hinB = hin

                    # ----- prefetch xw for step s+2 into the ring -----
                    nc.sync.dma_start(
                        xwr[(slot + 2) % NB][:].rearrange("p m n -> p (m n)"),
                        xw_d[bass.ds(ti + 2, 1), :, :].rearrange(
                            "t p c -> p (t c)"
                        ),
                    )

                    # ----- PE: xW inject (start=True clears each bank) -----
                    xw = xwr[slot]
                    mm(
                        sbIG[:].rearrange("p m n -> p (m n)"), ident_sb[:],
                        xw[:, 0:16, :].rearrange("p m n -> p (m n)"),
                        start=True,
                    )
                    mm(
                        sbF[:].rearrange("p m n -> p (m n)"), ident_sb[:],
                        xw[:, 16:24, :].rearrange("p m n -> p (m n)"),
                        start=True,
                    )
                    mm(
                        sbOa[:].rearrange("p m n -> p (m n)"), ident_sb[:],
                        xw[:, 24:28, :].rearrange("p m n -> p (m n)"),
                        start=True,
                    )
                    mm(
                        sbOb[:].rearrange("p m n -> p (m n)"), ident_sb[:],
                        xw[:, 28:32, :].rearrange("p m n -> p (m n)"),
                        start=True,
                    )

                    def hin_kc(kc):
                        return hinA[:, kc, :] if kc < H2 else hinB[:, kc - H2, :]

                    # ----- PE: scores (unblocks softmax ASAP) -----
                    for kc in range(HC):
                        nc.tensor.matmul(
                            scp[:],
                            afTb[:, kc, :, :].rearrange("p n l -> p (n l)"),
                            hin_kc(kc),
                            start=(kc == 0),
                            stop=(kc == HC - 1),
                        )

                    # ----- softmax chain at high scheduler priority: each op
                    # pops the moment it's ready, so the PE sem updates land
                    # right after scores/zsp instead of after the Wh burst
                    with tc.high_priority():
                        # V: diagonal-block extract s_col
                        junk = sp.tile([128, NL], FP32, tag="junk")
                        s_col = sp.tile([128, 1], FP32, tag="s_col")
                        nc.vector.scalar_tensor_tensor(
                            junk[:], scp[:], 1.0, mask_sb[:],
                            MULT, MULT,
                            accum_out=s_col[:],
                        )
                        # ACT: e^s = exp(4*u) in one Scalar op
                        e_col = sp.tile([128, 1], FP32, tag="e_col")
                        nc.scalar.activation(
                            e_col[:], s_col[:],
                            mybir.ActivationFunctionType.Exp, scale=4.0,
                        )
                        e_colb = sp.tile([128, 1], BF16, tag="e_colb")
                        nc.vector.tensor_copy(e_colb[:], e_col[:])


                    def wh_m(ms, kcs):
                        # kc-major: consecutive MMs accumulate into DIFFERENT
                        # PSUM addresses (same-address back-to-back accumulate
                        # costs ~4ns/MM in the drain path)
                        for kc in kcs:
                            for m in ms:
                                mm(
                                    sb_ap(m),
                                    wh_sb[:, kc, m * 128:(m + 1) * 128],
                                    hin_kc(kc),
                                )

                    def folds(ms):
                        for m in ms:
                            mm(
                                sb_ap(m),
                                p_sb[:, m * 128:(m + 1) * 128],
                                ee[:],
                                stop=True,
                            )

                    # i + g quarters: kc halves split so the first block only
                    # needs the first half of h from the previous step
                    wh_m(range(0, 16), range(0, H2))
                    # zsp placed here: ~2.5us into the Wh stream, safely after
                    # e_col resolves (~1.3us) -> never stalls the in-order PE
                    # queue, and ee is ready long before the folds' slot.
                    nc.tensor.matmul(
                        zsp[:], bmask_sb[:], e_colb[:],
                        start=True, stop=True,
                    )
                    with tc.high_priority():
                        rz = sp.tile([128, 1], FP32, tag="rz")
                        nc.vector.reciprocal(rz[:], zsp[:])
                        # ee = (mask * e_col) * (1/z)
                        ee = sp.tile([128, NL], BF16, tag="ee")
                        nc.vector.tensor_scalar(
                            ee[:], mask_sb[:], e_col[:, 0:1], rz[:, 0:1],
                            MULT, MULT,
                        )
                    wh_m(range(0, 16), range(H2, HC))
                    folds(range(0, 16))
                    folds(range(16, 24))
                    folds(range(24, 28))
                    folds(range(28, 32))

                    # ----- tail, pipelined by quarter group -----
                    glIG = sp.tile([128, 2, HC, NL], FP32, tag="glIG")
                    nc.scalar.activation(
                        glIG[:],
                        sbIG[:].rearrange("p (q c) n -> p q c n", q=2),
                        TANH,
                    )
                    gi = glIG[:, 0]
                    gg = glIG[:, 1]
                    # t2 = (gl_i + 1) * gl_g = 2 i g
                    t2 = sp.tile([128, HC, NL], FP32, tag="t2")
                    nc.vector.scalar_tensor_tensor(t2[:], gi, 1.0, gg, ADD, MULT)

                    wh_m(range(16, 24), range(0, HC))
                    glF = sp.tile([128, HC, NL], FP32, tag="glF")
                    nc.scalar.activation(glF[:], sbF[:], TANH)
                    cs = cT[:, :, :]
                    # u = (gl_f + 1) * cT(2c) = 4 f c
                    u = sp.tile([128, HC, NL], FP32, tag="u")
                    nc.vector.scalar_tensor_tensor(u[:], glF[:], 1.0, cs, ADD, MULT)
                    # cT = 0.5*u + t2 = 2 c_new
                    nc.vector.scalar_tensor_tensor(cs, u[:], 0.5, t2[:], MULT, ADD)
                    # tct = tanh(0.5 * cT) = tanh(c)   (before glO on Scalar)
                    tct = sp.tile([128, HC, NL], FP32, tag="tct")
                    nc.scalar.activation(tct[:], cs, TANH, scale=0.5)

                    # o quarter, split in two hc-halves: h2a lands early so
                    # the next step's Wh kc0-3 can start under this tail
                    houtA, houtB = hout
                    wh_m(range(24, 28), range(0, HC))
                    glOa = sp.tile([128, H2, NL], FP32, tag="glOa")
                    nc.scalar.activation(glOa[:], sbOa[:], TANH)
                    nc.vector.scalar_tensor_tensor(
                        houtA[:], glOa[:], 1.0, tct[:, 0:H2, :], ADD, MULT
                    )
                    wh_m(range(28, 32), range(0, HC))
                    glOb = sp.tile([128, H2, NL], FP32, tag="glOb")
                    nc.scalar.activation(glOb[:], sbOb[:], TANH)
                    nc.vector.scalar_tensor_tensor(
                        houtB[:], glOb[:], 1.0, tct[:, H2:, :], ADD, MULT
                    )
                    nc.sync.dma_start(
                        out_d[bass.ds(ti, 1), :, 0:H2, :].rearrange(
                            "t p c n -> p (t c) n"
                        ),
                        houtA[:],
                    )
                    nc.sync.dma_start(
                        out_d[bass.ds(ti, 1), :, H2:, :].rearrange(
                            "t p c n -> p (t c) n"
                        ),
                        houtB[:],
                    )

                with tc.For_i(
                    0, timesteps, UNROLL,
                    staggered_reset=True,
                    hint_engines=(ET.PE, ET.DVE, ET.SP, ET.Activation),
                ) as ti0:
                    for s in range(UNROLL):
                        step(
                            ti0 + s, s % NB,
                            hTbs[s % 4], hTbs[(s + 1) % 4],
                        )

    nc.finalize()
    return nc


def prep_inputs(x, A, Wx, Wh, Wattn, b):
    """Host-side reshapes to device layouts; returns per-core input maps."""
    x = np.asarray(x, dtype=np.float32)
    A = np.asarray(A, dtype=np.float32)
    Wx = np.asarray(Wx, dtype=np.float32)
    Wh = np.asarray(Wh, dtype=np.float32)
    Wattn = np.asarray(Wattn, dtype=np.float32)
    b = np.asarray(b, dtype=np.float32)
    timesteps = x.shape[1]

    # permute gate columns to quarter order [i, g, f, o]
    gperm = np.concatenate([
        np.arange(0, H),          # i
        np.arange(3 * H, 4 * H),  # g
        np.arange(H, 2 * H),      # f
        np.arange(2 * H, 3 * H),  # o
    ])
    Wx = Wx[:, gperm]
    Wh = Wh[:, gperm]
    Wattn = Wattn[:, gperm]
    b = b[gperm]

    # weight layouts [p, kc, g] with k = kc*128 + p
    # per-gate-column scaling: i/f/o columns carry a 0.5 (tanh half-angle
    # trick), g columns stay full-scale; Wh gets an extra 0.5 (h2 = 2h).
    gsc = np.ones((G,), np.float32) * 0.5
    gsc[H:2 * H] = 1.0            # g quarter (permuted position)
    whs = (0.5 * gsc) * Wh
    wxs = gsc * Wx
    wh_h = np.ascontiguousarray(
        whs.reshape(HC, 128, G).transpose(1, 0, 2).astype(ml_dtypes.bfloat16)
    )
    wx_h = np.ascontiguousarray(
        wxs.reshape(DC, 128, G).transpose(1, 0, 2).astype(ml_dtypes.bfloat16)
    )
    wattn_h = np.ascontiguousarray(
        (gsc * Wattn).reshape(HC, 128, G).transpose(1, 0, 2).astype(
            ml_dtypes.bfloat16
        )
    )
    # bias replicated across partitions; folded into P on device
    bp_h = np.ascontiguousarray(
        np.broadcast_to(gsc * b, (128, G)).astype(ml_dtypes.bfloat16)
    )
    mask_h = np.zeros((128, NL), dtype=np.float32)
    for p in range(128):
        mask_h[p, p // L] = 1.0
    bmask_h = (
        np.arange(128)[:, None] // L == np.arange(128)[None, :] // L
    ).astype(ml_dtypes.bfloat16)
    ident_h = np.eye(128, dtype=np.float32).astype(ml_dtypes.bfloat16)

    in_maps = []
    for c in range(NC):
        xs = x[c * NL:(c + 1) * NL]          # (8, T, 512)
        As = A[c * NL:(c + 1) * NL].reshape(NL, H, L)  # (8, 1024, 16)
        # xT [p, dc, n, t] = x[n, t, dc*128+p]
        xT_h = np.ascontiguousarray(
            xs.reshape(NL, timesteps, DC, 128).transpose(3, 2, 0, 1)
            .astype(ml_dtypes.bfloat16)
        )
        # afT [p, hc, n, l] = Af[n, hc*128+p, l]
        afT_h = np.ascontiguousarray(
            As.reshape(NL, HC, 128, L).transpose(2, 1, 0, 3)
        )
        in_maps.append(
            {
                "xT": xT_h,
                "afT": afT_h,
                "wx": wx_h,
                "wh": wh_h,
                "wattn": wattn_h,
                "biasp": bp_h,
                "mask": mask_h,
                "bmask": bmask_h,
                "ident": ident_h,
            }
        )
    return in_maps


_NC_CACHE = {}


def kernel(x, A, Wx, Wh, Wattn, b, trace=False):
    timesteps = x.shape[1]
    key = timesteps
    if key not in _NC_CACHE:
        _NC_CACHE[key] = build_nc(timesteps)
    nc = _NC_CACHE[key]
    in_maps = prep_inputs(x, A, Wx, Wh, Wattn, b)
    res = run_bass_kernel_spmd(nc, in_maps, list(range(NC)), trace=trace)
    outs = []
    for c in range(NC):
        hsT = res.results[c]["hsT"]  # (T, 128, HC, NL)
        # out[n, t, hc*128+p] = hsT[t, p, hc, n]
        outs.append(
            0.5
            * hsT.astype(np.float32).transpose(3, 0, 2, 1).reshape(
                NL, timesteps, H
            )
        )
    full = np.concatenate(outs, axis=0).astype(np.float32)
    kernel.last_result = res
    return full
